# revision 16
# baseline (speedup 1.0000x reference)
"""Multi-head attention (RoPE) Trainium2 kernel, 8-way sharded.

Sharding: core c handles batch b = c//4 and 4 heads h0 = 4*(c%4).

Per-core program (v7 — deep-pipelined boot + ACT-saturating schedule):
  Heads are processed as two pairs (0,1) and (2,3). Post-rope q/k live in
  pair tiles [128, 2048] with the even head in partitions 0-63 and the odd
  head in 64-127, so the two heads' score matmuls (K=64) run CONCURRENTLY
  as PE row-tiles (0,0)/(64,0) — 2x score throughput.

  PSUM (8 banks): sp0/sp1 score tiles [128,1024] f32 (2+2 banks, one per
  head, ping-ponged by the exp consumer), pvP [128,2,512] (2 banks, both
  heads' PV accumulators; row 64 = softmax denominator), and two 1-bank
  projection chains (A/B) for qkv/v/out-proj filler pieces.

  The ScalarE exp stream is the kernel floor: 128 ACTIVATE calls of
  (1024+~313)/1.2 ~= 1114 ns = 142.6 us.  Everything else is arranged to
  never stall it: boot is minimal (k0/q0 pieces + v0/v1 only; v2..v15 and
  remaining q/k pieces stream into the first windows as fillers), per-step
  emission keeps next-step score matmuls ahead of fillers, y-projection
  fillers are split to 1-matmul granularity across pair-1 windows, and
  always-ready low-priority keepalive matmuls prevent the PE HAM clock
  gate from dropping to 4/8 in any idle window.
  norm per (pair, qq): denominator row copied out of PSUM (custom-DVE ops
  must not read PSUM directly), one merged reciprocal for both heads,
  gpsimd broadcast, DVE mul.

  host: y[b] = sum of the 4 per-core partials (fp32).
"""

import numpy as np

B = 2
N = 2048
C = 1024
HD = 64
HC = 4  # heads per core
N_CORES = 8
ROPE_BASE = 10000.0

_PROGRAM = None
DEBUG = False


def _rope_tables():
    inv_freq = 1.0 / (ROPE_BASE ** (np.arange(0, HD, 2, dtype=np.float32) / HD))
    t = np.arange(N, dtype=np.float32)
    freqs = np.einsum("i,j->ij", t, inv_freq).astype(np.float32)  # [N, 32]
    emb = np.concatenate([freqs, freqs], axis=-1)  # [N, 64]
    cos = np.cos(emb).astype(np.float32)
    sin = np.sin(emb).astype(np.float32)
    cosT = np.ascontiguousarray(np.tile(cos.T, (2, 1)))  # [128, 2048]
    sinT = sin.T.copy()  # [64, 2048]
    sinT_signed = np.concatenate([-sinT[:32], sinT[32:]], axis=0)
    sinT2 = np.ascontiguousarray(np.tile(sinT_signed, (2, 1)))  # [128, 2048]
    return cosT, sinT2


def _fold(a, chunks):
    """[chunks*128, F] -> [128, chunks*F] partition-contiguous layout."""
    ch, rem = a.shape[0] // 128, a.shape[1]
    assert ch == chunks
    return np.ascontiguousarray(
        a.reshape(chunks, 128, rem).transpose(1, 0, 2).reshape(128, chunks * rem)
    )


def _build_program():
    import concourse.mybir as mybir
    import concourse.tile as tile
    from concourse import bacc

    f32 = mybir.dt.float32
    f16 = mybir.dt.float16
    MUL = mybir.AluOpType.mult
    ADD = mybir.AluOpType.add
    EXP = mybir.ActivationFunctionType.Exp

    nc = bacc.Bacc("TRN2", target_bir_lowering=False, debug=False, num_devices=N_CORES)

    xT_d = nc.dram_tensor("xTq", [128, 4 * 8 * 512], f16, kind="ExternalInput").ap()
    # wqk folded per qk-tile block: [128, (t, ct, 128)]
    wqk_d = nc.dram_tensor("wqkF", [128, 4 * 8 * 128], f16, kind="ExternalInput").ap()
    wv_d = nc.dram_tensor("wvF", [128, 8 * 256], f16, kind="ExternalInput").ap()
    wo_d = nc.dram_tensor("woF", [128, 2 * C], f16, kind="ExternalInput").ap()
    cos_d = nc.dram_tensor("cosT", [128, N], f32, kind="ExternalInput").ap()
    sin_d = nc.dram_tensor("sinT", [128, N], f32, kind="ExternalInput").ap()
    y_d = nc.dram_tensor("y", [N, C], f16, kind="ExternalOutput").ap()
    if DEBUG:
        ao_d = [
            nc.dram_tensor(f"aoD{p}", [128, N], f16, kind="ExternalOutput").ap()
            for p in range(2)
        ]
        dd_d = nc.dram_tensor("ddD", [1, 8 * 2 * 512], f32, kind="ExternalOutput").ap()
        rr_d = nc.dram_tensor("rrD", [1, 8 * 2 * 512], f32, kind="ExternalOutput").ap()

    with tile.TileContext(nc) as tc:
        with (
            tc.tile_pool(name="persist", bufs=1) as persist,
            tc.tile_pool(name="work", bufs=2) as work,
            tc.tile_pool(name="psum", bufs=1, space="PSUM") as psp,
        ):
            # ---------------- persistent SBUF ----------------
            xT = persist.tile([128, 4, 8, 512], f16, tag="xT", name="xT")
            wqk = persist.tile([128, 4, 8, 128], f16, tag="wqk", name="wqk")
            wv = persist.tile([128, 8, 256], f16, tag="wv", name="wv")
            wo = persist.tile([128, 2, C], f16, tag="wo", name="wo")
            cosT = persist.tile([128, N], f32, tag="cosT", name="cosT")
            sinT = persist.tile([128, N], f32, tag="sinT", name="sinT")
            # q-pair0, q-pair1, k-pair0, k-pair1  (matches wqkT col blocks)
            qk = [
                persist.tile([128, N], f16, tag=f"qk{t}", name=f"qk{t}")
                for t in range(4)
            ]
            vv = persist.tile([128, 16, HC, HD + 1], f16, tag="vv", name="vv")
            ao = [
                persist.tile([128, N], f16, tag=f"ao{p}", name=f"ao{p}")
                for p in range(2)
            ]
            dd = (
                persist.tile([1, 8, 2, 512], f32, tag="dd", name="dd")
                if DEBUG
                else None
            )
            rr = (
                persist.tile([1, 8, 2, 512], f32, tag="rr2", name="rr2")
                if DEBUG
                else None
            )

            # ---------------- helpers ----------------
            def dma_in():
                # earliest-needed-first: first qk pieces need wqk[t2]/[t0] +
                # x quarter 0; boot v0/v1 need wv; the early rope fillers
                # need the full cos/sin tables before x quarters 2/3.
                nc.sync.dma_start(wqk[:, 2], wqk_d[:, 2 * 1024 : 3 * 1024])
                nc.sync.dma_start(xT[:, 0, 0:4], xT_d[:, 0:2048])
                nc.sync.dma_start(xT[:, 0, 4:8], xT_d[:, 2048:4096])
                nc.sync.dma_start(wqk[:, 0], wqk_d[:, 0:1024])
                nc.sync.dma_start(cosT[:, 0:512], cos_d[:, 0:512])
                nc.sync.dma_start(sinT[:, 0:512], sin_d[:, 0:512])
                nc.sync.dma_start(wv[:], wv_d[:, :])
                nc.sync.dma_start(xT[:, 1], xT_d[:, 4096:8192])
                nc.sync.dma_start(xT[:, 2], xT_d[:, 8192:12288])
                nc.sync.dma_start(xT[:, 3], xT_d[:, 12288:16384])
                nc.sync.dma_start(cosT[:, 512:], cos_d[:, 512:])
                nc.sync.dma_start(sinT[:, 512:], sin_d[:, 512:])
                nc.sync.dma_start(wqk[:, 3], wqk_d[:, 3 * 1024 : 4 * 1024])
                nc.sync.dma_start(wqk[:, 1], wqk_d[:, 1 * 1024 : 2 * 1024])
                nc.sync.dma_start(wo[:], wo_d[:, :])

            def pe_warmup():
                # ~8 junk matmuls on a zeroed scratch so the PE HAM clock
                # gate is at 8/8 by the time the real pipeline starts
                wsc = work.tile([128, 512], f16, tag="wsc", name="wsc")
                nc.vector.memset(wsc[:], 0.0)
                wps = psp.tile([128, 512], f32, tag="sp0", name="wps")
                for r in range(8):
                    nc.tensor.matmul(
                        wps[:], wsc[:, 0:128], wsc[:], start=(r == 0), stop=(r == 7)
                    )

            def act_table_preload():
                scratch = work.tile([128, 16], f32, tag="dmy", name="dmy")
                nc.vector.memset(scratch[:], 0.0)
                dmye = work.tile([128, 16], f16, tag="dmye", name="dmye")
                nc.scalar.activation(dmye[:], scratch[:], EXP)

            def qk_piece(t, pc, chain, part=None):
                """project + rope one 512-token piece of qk tile t.
                part=None: whole piece; part=(state, 0/1): half for smooth
                filler interleave (4 matmuls per half, rope with part 1)."""
                sl = slice(pc * 512, (pc + 1) * 512)
                if part is None or part[1] == 0:
                    bp = psp.tile([128, 512], f32, tag=chain, name=f"bp{t}_{pc}")
                    if part is not None:
                        part[0]["bp"] = bp
                else:
                    bp = part[0]["bp"]
                cts = range(8) if part is None else (
                    range(4) if part[1] == 0 else range(4, 8)
                )
                for ct in cts:
                    nc.tensor.matmul(
                        bp[:],
                        wqk[:, t, ct, :],
                        xT[:, pc, ct, :],
                        start=(ct == 0),
                        stop=(ct == 7),
                    )
                if part is not None and part[1] == 0:
                    return
                t_sb = work.tile([128, 512], f32, tag="ropet", name="rt")
                u_sb = work.tile([128, 512], f32, tag="ropeu", name="ru")
                nc.vector.tensor_tensor(t_sb[:], bp[:], cosT[:, sl], MUL)
                for o_lo, i_lo in [(0, 32), (32, 0), (64, 96), (96, 64)]:
                    nc.vector.tensor_tensor(
                        u_sb[o_lo : o_lo + 32, :],
                        bp[i_lo : i_lo + 32, :],
                        sinT[o_lo : o_lo + 32, sl],
                        MUL,
                    )
                nc.vector.tensor_tensor(qk[t][:, sl], t_sb[:], u_sb[:], ADD)

            # filler schedule: maps (pair, qq, i) -> list of thunks.
            fillers = {}

            def add_fill(pair, qq, i, fn):
                fillers.setdefault((pair, qq, i), []).append(fn)

            def fill_qk2(pair_qq_i_a, pair_qq_i_b, t, pc, chain):
                """schedule one qk piece as two 4-matmul halves."""
                st = {}
                add_fill(*pair_qq_i_a, lambda: qk_piece(t, pc, chain, (st, 0)))
                add_fill(*pair_qq_i_b, lambda: qk_piece(t, pc, chain, (st, 1)))

            def v_piece(tt, chain, on_scalar=False, part=None):
                """V' tile for one 128-token block (token-major)."""
                if part is None or part[1] == 0:
                    vp = psp.tile([128, 256], f32, tag=chain, name=f"vp{tt}")
                    if part is not None:
                        part[0]["vp"] = vp
                else:
                    vp = part[0]["vp"]
                tsl = slice((tt % 4) * 128, (tt % 4) * 128 + 128)
                cts = range(8) if part is None else (
                    range(4) if part[1] == 0 else range(4, 8)
                )
                for ct in cts:
                    nc.tensor.matmul(
                        vp[:],
                        xT[:, tt // 4, ct, tsl],
                        wv[:, ct, :],
                        start=(ct == 0),
                        stop=(ct == 7),
                    )
                if part is not None and part[1] == 0:
                    return
                dst = vv[:, tt, :, 0:HD]
                srcap = vp[:].rearrange("p (h d) -> p h d", h=HC)
                if on_scalar:
                    nc.scalar.copy(dst, srcap)
                else:
                    nc.vector.tensor_copy(dst, srcap)

            def fill_v2(a, b, tt, chain, on_scalar=False):
                st = {}
                add_fill(*a, lambda: v_piece(tt, chain, on_scalar, part=(st, 0)))
                add_fill(*b, lambda: v_piece(tt, chain, on_scalar, part=(st, 1)))

            def y_piece(tt, oc, chain, on_scalar=False, part=None):
                osl = slice(oc * 512, (oc + 1) * 512)
                if part is None or part[1] == 0:
                    yps = psp.tile([128, 512], f32, tag=chain, name=f"yps{tt}_{oc}")
                    if part is not None:
                        part[0]["yps"] = yps
                else:
                    yps = part[0]["yps"]
                ps = range(2) if part is None else [part[1]]
                for p in ps:
                    nc.tensor.matmul(
                        yps[:],
                        ao[p][:, tt * 128 : (tt + 1) * 128],
                        wo[:, p, osl],
                        start=(p == 0),
                        stop=(p == 1),
                    )
                if part is not None and part[1] == 0:
                    return
                ysb = work.tile([128, 512], f16, tag="ysb", bufs=3, name="ysb")
                if on_scalar:
                    nc.scalar.copy(ysb[:], yps[:])
                    nc.scalar.dma_start(y_d[tt * 128 : (tt + 1) * 128, osl], ysb[:])
                else:
                    nc.vector.tensor_copy(ysb[:], yps[:])
                    nc.sync.dma_start(y_d[tt * 128 : (tt + 1) * 128, osl], ysb[:])

            def fill_y2(a, b, tt, oc, chain):
                st = {}
                add_fill(*a, lambda: y_piece(tt, oc, chain, part=(st, 0)))
                add_fill(*b, lambda: y_piece(tt, oc, chain, part=(st, 1)))

            def keepalive(chain, n=2):
                # always-ready junk matmuls at very late priority: the
                # scheduler only places them where the PE would otherwise
                # idle, keeping the HAM clock gate at 8/8.
                with tc.high_priority(offset=-(1 << 20)):
                    kps = psp.tile([128, 512], f32, tag=chain, name="ka")
                    for r in range(n):
                        nc.tensor.matmul(
                            kps[:], wsc2[:, 0:128], wsc2[:],
                            start=(r == 0), stop=(r == n - 1),
                        )

            # ---- window (0,0): all projection work is in boot; keepalives
            for _i in range(8):
                add_fill(0, 0, _i, (lambda ch: lambda: keepalive(ch, 2))(
                    "pA" if _i % 2 == 0 else "pB"))
            # ---- window (0,1): q0 qq2, k-pair1 pc0/1, q1 qq0
            fill_qk2((0, 1, 0), (0, 1, 1), 0, 2, "pA")
            fill_qk2((0, 1, 2), (0, 1, 3), 3, 0, "pB")
            fill_qk2((0, 1, 4), (0, 1, 5), 3, 1, "pA")
            fill_qk2((0, 1, 6), (0, 1, 7), 1, 0, "pB")
            # ---- window (0,2): q0 qq3, k-pair1 pc2/3
            fill_qk2((0, 2, 0), (0, 2, 1), 0, 3, "pA")
            fill_qk2((0, 2, 2), (0, 2, 3), 3, 2, "pB")
            fill_qk2((0, 2, 4), (0, 2, 5), 3, 3, "pA")
            add_fill(0, 2, 6, lambda: keepalive("pB", 2))
            add_fill(0, 2, 7, lambda: keepalive("pB", 2))
            # ---- window (0,3): q1 qq1
            fill_qk2((0, 3, 0), (0, 3, 1), 1, 1, "pA")
            add_fill(0, 3, 2, lambda: keepalive("pB", 2))
            add_fill(0, 3, 4, lambda: keepalive("pB", 2))
            add_fill(0, 3, 6, lambda: keepalive("pA", 2))
            add_fill(0, 3, 7, lambda: keepalive("pA", 2))
            # ---- window (1,0): q1 qq2/qq3 (real PE work in the HAM hole)
            fill_qk2((1, 0, 0), (1, 0, 1), 1, 2, "pA")
            fill_qk2((1, 0, 2), (1, 0, 3), 1, 3, "pB")
            add_fill(1, 0, 4, lambda: keepalive("pA", 2))
            add_fill(1, 0, 5, lambda: keepalive("pB", 2))
            add_fill(1, 0, 6, lambda: keepalive("pA", 2))
            add_fill(1, 0, 7, lambda: keepalive("pB", 2))
            # ---- windows (1,1..3): out-proj halves for previous qq's blocks
            for qq in range(1, 4):
                for j in range(4):
                    tt = (qq - 1) * 4 + j
                    i0 = 2 * j
                    fill_y2((1, qq, i0), (1, qq, i0 + 1), tt, 0, "pA")
                    fill_y2((1, qq, i0), (1, qq, i0 + 1), tt, 1, "pB")
                add_fill(1, qq, 1, lambda: keepalive("pA", 2))
                add_fill(1, qq, 3, lambda: keepalive("pB", 2))
                add_fill(1, qq, 5, lambda: keepalive("pA", 2))
                add_fill(1, qq, 7, lambda: keepalive("pB", 2))

            def norm(pair, qq, pvP):
                # ao[pair][j*64:(j+1)*64, qsl] = pvP[0:64, j] / pvP[64, j]
                # (den must be copied out of PSUM first: custom-DVE ops
                # reading PSUM directly corrupt scattered lanes)
                qsl = slice(qq * 512, (qq + 1) * 512)
                den = work.tile([1, 2, 512], f32, tag="den", name="den")
                nc.vector.tensor_copy(den[:], pvP[HD : HD + 1, :, :])
                pvS = work.tile([HD, 2, 512], f32, tag="pvS", name="pvS")
                nc.vector.tensor_copy(pvS[:], pvP[0:HD, :, :])
                ra = work.tile([1, 2, 512], f32, tag="ra", name="ra")
                nc.vector.reciprocal_approx_fast(ra[:], den[:])
                if DEBUG:
                    g = pair * 4 + qq
                    nc.vector.tensor_copy(dd[0:1, g, :, :], den[:])
                    nc.vector.tensor_copy(rr[0:1, g, :, :], ra[:])
                for j in (0, 1):
                    nb = work.tile([64, 512], f32, tag="nb", name="nb")
                    nc.gpsimd.partition_broadcast(nb[:], ra[0:1, j, :])
                    nc.vector.tensor_tensor(
                        ao[pair][j * 64 : (j + 1) * 64, qsl],
                        pvS[:, j, :],
                        nb[:],
                        MUL,
                    )

            def emit_S_head(pair, qq, i, which):
                """score matmuls for one head of (pair, qq, ktpair).
                which=0: even head -> sp0 (PE rows 0-63); which=1: odd -> sp1."""
                qt = qk[pair]
                kt_ = qk[2 + pair]
                qsl = slice(qq * 512, (qq + 1) * 512)
                lo, hi = (0, 64) if which == 0 else (64, 128)
                with tc.high_priority():
                    sp = psp.tile(
                        [128, 1024], f32, tag=f"sp{which}",
                        name=f"sp{which}_{pair}_{qq}_{i}",
                    )
                    for half, kk in ((0, 2 * i), (1, 2 * i + 1)):
                        ksl = slice(kk * 128, (kk + 1) * 128)
                        ssl = slice(half * 512, (half + 1) * 512)
                        nc.tensor.matmul(
                            sp[:, ssl], kt_[lo:hi, ksl], qt[lo:hi, qsl],
                            start=True, stop=True,
                        )
                return sp

            def attention(pairs):
                # software-pipelined: score matmuls one step ahead; next-step
                # sp matmuls always precede fillers in priority so the exp
                # stream never queues behind filler work.
                steps = [
                    (pair, qq, i) for pair in pairs for qq in range(4) for i in range(8)
                ]
                sp0 = emit_S_head(*steps[0], 0)
                sp1 = emit_S_head(*steps[0], 1)
                pvs = {}
                for n, (pair, qq, i) in enumerate(steps):
                    if i == 0:
                        pvs[pair, qq] = psp.tile(
                            [128, 2, 512], f32, tag="pvP", name=f"pvP_{pair}_{qq}"
                        )
                    pvP = pvs[pair, qq]
                    es0 = work.tile([128, 1024], f16, tag="es0", bufs=3, name="es0")
                    es1 = work.tile([128, 1024], f16, tag="es1", bufs=3, name="es1")
                    with tc.high_priority():
                        nc.scalar.activation(
                            es0[:], sp0[:], EXP, scale=float(HD**-0.5)
                        )
                        nc.scalar.activation(
                            es1[:], sp1[:], EXP, scale=float(HD**-0.5)
                        )
                    nxt = steps[n + 1] if n + 1 < len(steps) else None

                    def emit_pv(j, es):
                        for half, kk in ((0, 2 * i), (1, 2 * i + 1)):
                            nc.tensor.matmul(
                                pvP[0 : HD + 1, j, :],
                                vv[:, kk, 2 * pair + j, :],
                                es[:, half * 512 : (half + 1) * 512],
                                start=(i == 0 and half == 0),
                                stop=(i == 7 and half == 1),
                            )

                    if i == 0:
                        # boundary: first PV matmuls wait for the previous
                        # quarter's norm (pvP WAR); emit both S heads and
                        # the fillers first so the exp stream and the next
                        # iter's scores aren't head-blocked behind them
                        if nxt:
                            sp0 = emit_S_head(*nxt, 0)
                            sp1 = emit_S_head(*nxt, 1)
                        for fn in fillers.get((pair, qq, i), ()):
                            fn()
                        emit_pv(0, es0)
                        emit_pv(1, es1)
                    else:
                        if nxt:
                            sp0 = emit_S_head(*nxt, 0)
                        emit_pv(0, es0)
                        if nxt:
                            sp1 = emit_S_head(*nxt, 1)
                        emit_pv(1, es1)
                        if i == 7:
                            norm(pair, qq, pvP)
                        for fn in fillers.get((pair, qq, i), ()):
                            fn()

            # ---------------- emission ----------------
            act_table_preload()
            pe_warmup()
            wsc2 = persist.tile([128, 512], f16, tag="wsc2", name="wsc2")
            nc.vector.memset(wsc2[:], 0.0)
            dma_in()
            nc.vector.memset(vv[:, :, :, HD : HD + 1], 1.0)
            # boot: k-pair0 cols 0-1023, q-pair0 cols 0-511, v blocks 0-13.
            # Six independent psum tags in dependency-ready emission order;
            # v-copies go to the idle ScalarE so the DVE queue holds only
            # the boot rope ops.
            qk_piece(2, 0, "pB")
            qk_piece(0, 0, "pA")
            v_piece(0, "pB", True)
            v_piece(1, "pA", True)
            qk_piece(2, 1, "pB")
            qk_piece(0, 1, "pA")
            v_piece(2, "pB", True)
            v_piece(3, "pA", True)
            qk_piece(2, 2, "pB")
            v_piece(4, "pA", True)
            v_piece(5, "pB", True)
            qk_piece(2, 3, "pB")
            v_piece(6, "pA", True)
            v_piece(7, "pB", True)
            v_piece(8, "pA", True)
            v_piece(9, "pB", True)
            v_piece(10, "pA", True)
            v_piece(11, "pB", True)
            v_piece(12, "pA", True)
            v_piece(13, "pB", True)
            v_piece(14, "pA", True)
            v_piece(15, "pB", True)

            attention((0, 1))

            # tail keepalives: keep the PE clock warm from last exp to the
            # tail out-proj matmuls
            for _ in range(8):
                keepalive("pvP", 3)

            # tail: out-proj for the last quarter's token blocks, 4 chains,
            # evac copies split across VectorE and the now-idle ScalarE
            for j, chain, on_sc in (
                (0, "pA", False),
                (1, "pB", False),
                (2, "sp0", True),
                (3, "sp1", True),
            ):
                tt = 12 + j
                y_piece(tt, 0, chain, on_sc)
                y_piece(tt, 1, chain, on_sc)

            if DEBUG:
                for p in range(2):
                    nc.sync.dma_start(ao_d[p][:, :], ao[p][:])
                nc.sync.dma_start(dd_d[:, :], dd[:])
                nc.sync.dma_start(rr_d[:, :], rr[:])

    nc.compile()
    return nc


def _get_program():
    global _PROGRAM
    if _PROGRAM is None:
        _PROGRAM = _build_program()
    return _PROGRAM


def _make_in_maps(x, w_qkv, w_out):
    x = np.asarray(x, dtype=np.float32)
    w_qkv = np.asarray(w_qkv, dtype=np.float32)
    w_out = np.asarray(w_out, dtype=np.float32)
    cosT, sinT = _rope_tables()
    in_maps = []
    for c in range(N_CORES):
        b = c // 4
        h0 = HC * (c % 4)
        rows = np.arange(h0 * HD, (h0 + HC) * HD)
        wq = w_qkv[rows]  # [256, 1024]
        wk = w_qkv[C + rows]
        wv = w_qkv[2 * C + rows]
        xT_c = np.ascontiguousarray(x[b].T).astype(np.float16)  # [1024, 2048]
        # xT quarters folded: [128, (pc, ct, f)]
        xTq = np.ascontiguousarray(
            xT_c.reshape(8, 128, 4, 512).transpose(1, 2, 0, 3).reshape(128, -1)
        )
        wqkT = np.concatenate([wq, wk], 0).T.astype(np.float16)  # [1024, 512]
        # wqk folded per qk-tile block: [128, (t, ct, 128)]
        wqkF = np.ascontiguousarray(
            wqkT.reshape(8, 128, 4, 128).transpose(1, 2, 0, 3).reshape(128, -1)
        )
        woT = w_out[:, rows].T.astype(np.float16)  # [256, 1024]
        in_maps.append(
            {
                "xTq": xTq,
                "wqkF": wqkF,
                "wvF": _fold(wv.T.astype(np.float16), 8),
                "woF": _fold(woT, 2),
                "cosT": cosT,
                "sinT": sinT,
            }
        )
    return in_maps


def run(inputs, trace=False, trace_cores=None):
    from concourse.bass_utils import run_bass_kernel_spmd

    nc = _get_program()
    in_maps = _make_in_maps(inputs["x"], inputs["w_qkv"], inputs["w_out"])
    res = run_bass_kernel_spmd(
        nc,
        in_maps,
        core_ids=list(range(N_CORES)),
        trace=trace,
        trace_cores=trace_cores,
    )
    y = np.zeros((B, N, C), dtype=np.float32)
    for c in range(N_CORES):
        y[c // 4] += res.results[c]["y"].astype(np.float32)
    return y, res


def kernel(**inputs) -> np.ndarray:
    y, _ = run(inputs, trace=False)
    return y
